# revision 15
# baseline (speedup 1.0000x reference)
"""Sparse (graph-masked) multi-head attention on 8 Trainium2 NeuronCores.

Reference computation (fp32, single device):
    qkv = x @ w_qkv + b_qkv ; split heads (H=8, D=64)
    scores = q k^T / sqrt(D), masked by adj_matrix (True=attend)
    y = softmax(scores) @ v ; out = y @ w_proj + b_proj

Sharding: core = (batch b, query-half th).  Each core owns queries
t in [th*1024, (th+1)*1024) of batch b and produces out[b, that slice, :].
No cross-core communication.

Engine strategy (v2): the kernel is elementwise-bound (exp on ACT,
mask-mul on DVE over H*TL*T = 16.8M elements/core), so:
  * 13/16 key-chunks: ACT exp (with a mean-compensation bias, see below)
    then DVE mask-multiply (bf16 2x mode, broadcast AP).
  * 3/16 key-chunks (DVE_I): one fused DVE scalar_tensor_tensor computes
    round(scores*A + maskS) -> int16, whose bits ARE the bf16 of
    g*exp(scores/sqrt(D)) masked (Schraudolph exponent trick).  maskS
    rows hold 16256 (=127<<7, attend) or 4096 (masked -> 2^-95).
    The common factor g=2^E[log2((1+f)/2^f)] is applied to the ACT
    chunks as exp bias ln(g) so both paths are mean-consistent; g
    cancels row-wise in softmax.
  * softmax denominators: v_aug has a per-head ones-column at local
    column h, so head h's denominator accumulates on PSUM lane h while
    y occupies lanes 8:72.  One [72,TB] copy evacuates y+d together;
    tiny DMAs gather all 8 d-rows into one [8,TL] tile for a single
    batched DVE reciprocal (vs 16 serial [1,512] reciprocals).
  * q/k PSUM evac on ACT (Identity + per-partition bias AP) to offload
    DVE; v evac keeps the fused bias stt on DVE.
  * phase-1 matmuls are interleaved into early attention pairs' PE idle
    slots; PSUM: psA 2x2 banks + psY 2x1 + ps1 2x1 = 8 banks.
"""

import numpy as np
import ml_dtypes

import concourse.bass as bass
import concourse.mybir as mybir
import concourse.tile as tile
from concourse import bacc
from concourse.bass_utils import run_bass_kernel_spmd

BF16 = mybir.dt.bfloat16
F32 = mybir.dt.float32
I16 = mybir.dt.int16
nbf16 = ml_dtypes.bfloat16

B, T, C, H = 4, 2048, 512, 8
D = C // H          # 64
P = 128
NCORES = 8
TL = T // 2         # queries per core
SCALE = 1.0 / float(np.sqrt(D))
LOG2E = float(np.log2(np.e))

# Schraudolph constants (bf16 = top 16 bits of f32; 7 mantissa bits)
SCH_A = SCALE * LOG2E * 128.0          # scores * A + maskS -> i16 bits
SCH_ON = 16256.0                       # 127 << 7 (attend)
SCH_OFF = 4096.0                       # -> 2^-95 ~ 0   (masked)
# mean multiplicative bias of the linear-mantissa approx: g = 2^c
SCH_C = 0.05730496                     # E[log2((1+f)/2^f)]
ACT_BIAS = float(np.log(2.0 ** SCH_C))  # exp(x + ln g) on ACT chunks
DVE_I = (13, 14, 15)                   # s-chunks computed on DVE

AF = mybir.ActivationFunctionType
ALU = mybir.AluOpType

VW = D + 8          # v columns per head: 64 dims + 8 ones-slot lanes
NB = 512            # one PSUM bank of f32
TB = 512            # t-block per attention pass


def build_program(t_full=T, t_local=TL, loop_reps=1, num_devices=NCORES):
    nkc = C // P                # contraction chunks over C
    nsc = t_full // P           # key/s chunks
    ntc = t_local // P          # output t chunks
    ntb = t_local // TB

    nc = bacc.Bacc("TRN2", target_bir_lowering=False, debug=False,
                   num_devices=num_devices)

    xT = nc.dram_tensor("xT", [C, t_full], BF16, kind="ExternalInput").ap()
    xTq = nc.dram_tensor("xTq", [C, t_local], BF16, kind="ExternalInput").ap()
    maskT = nc.dram_tensor("maskT", [t_full, t_local], BF16,
                           kind="ExternalInput").ap()
    wq = nc.dram_tensor("wq", [C, C], BF16, kind="ExternalInput").ap()
    wk = nc.dram_tensor("wk", [C, C], BF16, kind="ExternalInput").ap()
    wv = nc.dram_tensor("wv", [C, C], BF16, kind="ExternalInput").ap()
    wp = nc.dram_tensor("wp", [C, C], BF16, kind="ExternalInput").ap()
    bq = nc.dram_tensor("bq", [C], F32, kind="ExternalInput").ap()
    bk = nc.dram_tensor("bk", [C], F32, kind="ExternalInput").ap()
    bp = nc.dram_tensor("bp", [1, C], F32, kind="ExternalInput").ap()
    out = nc.dram_tensor("out", [t_local, C], F32, kind="ExternalOutput").ap()

    with tile.TileContext(nc) as tc:
        def body():
            with (tc.tile_pool(name="persist", bufs=1) as pp,
                  tc.tile_pool(name="psA", bufs=3, space="PSUM") as psA,
                  tc.tile_pool(name="psY", bufs=1, space="PSUM") as psY):
                # ---- input loads ----
                xT_sb = pp.tile([P, nkc, t_full], BF16, tag="xT")
                nc.sync.dma_start(
                    xT_sb[:], xT.rearrange("(k p) t -> p k t", p=P))
                xTq_sb = pp.tile([P, nkc, t_local], BF16, tag="xTq")
                nc.sync.dma_start(
                    xTq_sb[:], xTq.rearrange("(k p) t -> p k t", p=P))
                mask_sb = pp.tile([P, nsc, t_local], BF16, tag="mask")
                mask_r = maskT.rearrange("(i p) t -> p i t", p=P)
                ngrp = min(4, nsc)
                for g in range(ngrp):
                    gs = nsc // ngrp
                    nc.gpsimd.dma_start(mask_sb[:, g * gs:(g + 1) * gs],
                                        mask_r[:, g * gs:(g + 1) * gs])
                w_sb = {}
                for name, w in (("wq", wq), ("wk", wk), ("wv", wv),
                                ("wp", wp)):
                    w_sb[name] = pp.tile([P, nkc, C], BF16, tag=name,
                                         name=name)
                    nc.sync.dma_start(
                        w_sb[name][:], w.rearrange("(k p) c -> p k c", p=P))
                bq_sb = pp.tile([P, nkc], F32, tag="bq")
                nc.sync.dma_start(bq_sb[:], bq.rearrange("(j p) -> p j", p=P))
                bk_sb = pp.tile([P, nkc], F32, tag="bk")
                nc.sync.dma_start(bk_sb[:], bk.rearrange("(j p) -> p j", p=P))
                bp_row = pp.tile([1, C], F32, tag="bp_row")
                nc.sync.dma_start(bp_row[:], bp[:])
                bp_bc = pp.tile([P, C], F32, tag="bp_bc")
                nc.gpsimd.partition_broadcast(bp_bc[:], bp_row[:])

                # ---- persistent compute tiles ----
                qT_sb = pp.tile([P, nkc, t_local], BF16, tag="qT")
                kT_sb = pp.tile([P, nkc, t_full], BF16, tag="kT")
                v_sb = pp.tile([P, nsc, H * VW], BF16, tag="v")
                yu = [pp.tile([72, t_local], BF16, tag=f"yu{h}",
                              name=f"yu{h}") for h in range(H)]
                yT_pair = [pp.tile([P, t_local], BF16, tag=f"yTp{j}",
                                   name=f"yTp{j}") for j in range(H // 2)]
                d_all = pp.tile([8, t_local], BF16, tag="d_all")
                d_recip = pp.tile([8, t_local], BF16, tag="d_recip")
                d_row = [pp.tile([1, TB], BF16, tag=f"dr{h}",
                                 name=f"dr{h}") for h in range(H)]
                rbc = pp.tile([64, H, TB], BF16, tag="rbc")
                cbias = pp.tile([P, 1], F32, tag="cbias")
                nc.vector.memset(cbias[:], ACT_BIAS)

                # v_aug ones/zero slots: head h occupies v_sb cols
                # [VW*h, VW*(h+1)); local cols 0:64 are the v dims, cols
                # 64:72 d-slots (ones at local col 64+h -> PSUM lane 64+h;
                # engine ops need 32-aligned partition bases, so y keeps
                # lanes 0:64 and d sits above).
                vz = v_sb[:].rearrange("p i (h w) -> p i h w", h=H, w=VW)
                nc.vector.memset(vz[:, :, :, D:VW], 0.0)
                for h in range(H):
                    nc.vector.memset(
                        vz[:, :, h, D + h:D + h + 1], 1.0)

                # ---- phase-1 emitters: one [128, 2*TB] psA tile each,
                # evacuated by a single ACT op (Identity + bias AP) ----
                def emit_q(j):
                    pq = psA.tile([P, 2 * TB], F32, tag="s", name="pq")
                    for sl_i in range(2):
                        sl = slice(sl_i * NB, (sl_i + 1) * NB)
                        for k in range(nkc):
                            nc.tensor.matmul(
                                pq[:, sl],
                                w_sb["wq"][:, k, j * P:(j + 1) * P],
                                xTq_sb[:, k, sl],
                                start=(k == 0), stop=(k == nkc - 1))
                    nc.scalar.activation(qT_sb[:, j, :], pq[:], AF.Identity,
                                         bias=bq_sb[:, j:j + 1], scale=1.0)

                def emit_k(j, half):
                    pk = psA.tile([P, 2 * TB], F32, tag="s", name="pk")
                    base = half * 2 * NB
                    for sl_i in range(2):
                        sl = slice(sl_i * NB, (sl_i + 1) * NB)
                        gsl = slice(base + sl_i * NB, base + (sl_i + 1) * NB)
                        for k in range(nkc):
                            nc.tensor.matmul(
                                pk[:, sl],
                                w_sb["wk"][:, k, j * P:(j + 1) * P],
                                xT_sb[:, k, gsl],
                                start=(k == 0), stop=(k == nkc - 1))
                    nc.scalar.activation(
                        kT_sb[:, j, base:base + 2 * NB], pk[:], AF.Identity,
                        bias=bk_sb[:, j:j + 1], scale=1.0)

                def emit_v(i):
                    # v for s-chunks i, i+1 in one tile; pure copy evac
                    pv = psA.tile([P, 2 * TB], F32, tag="s", name="pv")
                    for a in range(2):
                        sl = slice(a * NB, (a + 1) * NB)
                        for k in range(nkc):
                            nc.tensor.matmul(
                                pv[:, sl],
                                xT_sb[:, k, (i + a) * P:(i + a + 1) * P],
                                w_sb["wv"][:, k], start=(k == 0),
                                stop=(k == nkc - 1))
                    v_dst = v_sb[:, i:i + 2].rearrange(
                        "p a (h w) -> p a h w", w=VW)[:, :, :, 0:D]
                    nc.scalar.activation(
                        v_dst, pv[:].rearrange("p (a h d) -> p a h d",
                                               a=2, d=D),
                        AF.Identity, bias=0.0, scale=1.0)

                # ---- lead-in: what attention pair 0 needs first ----
                emit_q(0)
                for half in range(2):
                    emit_k(0, half)
                emit_v(0)

                # extras: remaining phase-1 interleaved into pairs 0/1 of
                # tb 0, one item per chunk iteration (pair p's q/k are
                # emitted during earlier pairs).
                extras = {
                    (0, 0): ([lambda i=i: emit_v(i)
                              for i in range(2, nsc, 2)]
                             + [lambda: emit_q(1),
                                lambda: emit_k(1, 0), lambda: emit_k(1, 1)]),
                    (0, 1): [lambda: emit_q(2),
                             lambda: emit_k(2, 0), lambda: emit_k(2, 1),
                             lambda: emit_q(3),
                             lambda: emit_k(3, 0), lambda: emit_k(3, 1)],
                }

                # ---- phase 2: attention (tb outer, pair inner) ----
                for tb in range(ntb):
                    tsl = slice(tb * TB, (tb + 1) * TB)
                    for pair in range(H // 2):
                        h0, h1 = 2 * pair, 2 * pair + 1
                        todo = list(extras.get((tb, pair), []))
                        py0 = psY.tile([72, TB], F32, tag="py0", name="py0")
                        py1 = psY.tile([72, TB], F32, tag="py1", name="py1")
                        ps_next = {}

                        def emit_scores(i):
                            ps = psA.tile([P, 2 * TB], F32, tag="s",
                                          name="ps")
                            ps_next[i] = ps
                            nc.tensor.matmul(
                                ps[:, 0:TB],
                                kT_sb[0:D, pair, i * P:(i + 1) * P],
                                qT_sb[0:D, pair, tsl],
                                start=True, stop=True, tile_position=(0, 0))
                            nc.tensor.matmul(
                                ps[:, TB:2 * TB],
                                kT_sb[D:P, pair, i * P:(i + 1) * P],
                                qT_sb[D:P, pair, tsl],
                                start=True, stop=True, tile_position=(D, 0))

                        # scores run AHEAD chunks ahead of consumption;
                        # 1 when phase-1 extras share the psA pool, else 2.
                        ahead = 1 if todo else 2
                        emitted = [0]

                        def pump(upto):
                            while emitted[0] <= min(upto, nsc - 1):
                                emit_scores(emitted[0])
                                emitted[0] += 1

                        pump(ahead - 1)
                        for i in range(nsc):
                            pump(i + ahead)
                            # one interleaved phase-1 item per chunk
                            if todo:
                                todo.pop(0)()
                            ps = ps_next.pop(i)
                            if i in DVE_I:
                                maskS_bc = mask_sb[:, i, tsl].rearrange(
                                    "p (o n) -> p o n",
                                    o=1).broadcast_to([P, 2, TB])
                                am = pp_am(tc, pair, tb, i)
                                nc.vector.scalar_tensor_tensor(
                                    am[:].bitcast(I16).rearrange(
                                        "p (g n) -> p g n", g=2),
                                    ps[:].rearrange("p (g n) -> p g n", g=2),
                                    SCH_A, maskS_bc,
                                    op0=ALU.mult, op1=ALU.add)
                            else:
                                at = pp_at(tc, pair, tb, i)
                                nc.scalar.activation(at[:], ps[:], AF.Exp,
                                                     scale=SCALE,
                                                     bias=cbias[:, 0:1])
                                mask_bc = mask_sb[:, i, tsl].rearrange(
                                    "p (o n) -> p o n",
                                    o=1).broadcast_to([P, 2, TB])
                                am = pp_am(tc, pair, tb, i)
                                nc.vector.tensor_mul(
                                    am[:].rearrange("p (g n) -> p g n", g=2),
                                    at[:].rearrange("p (g n) -> p g n", g=2),
                                    mask_bc)
                            nc.tensor.matmul(
                                py0[:], v_sb[:, i, h0 * VW:(h0 + 1) * VW],
                                am[:, 0:TB], start=(i == 0),
                                stop=(i == nsc - 1))
                            nc.tensor.matmul(
                                py1[:], v_sb[:, i, h1 * VW:(h1 + 1) * VW],
                                am[:, TB:2 * TB], start=(i == 0),
                                stop=(i == nsc - 1))
                        while todo:
                            todo.pop(0)()
                        # evacuate y (lanes 8:72) + d (lane h) together
                        nc.vector.tensor_copy(yu[h0][:, tsl], py0[:])
                        nc.vector.tensor_copy(yu[h1][:, tsl], py1[:])

                    # ---- per-tb: batched reciprocal + normalize ----
                    for h in range(H):
                        nc.gpsimd.dma_start(d_all[h:h + 1, tsl],
                                            yu[h][D + h:D + h + 1, tsl])
                    with nc.allow_low_precision(
                            reason="softmax denom ~1e3, bf16 ok"):
                        nc.vector.reciprocal(d_recip[:, tsl],
                                             d_all[:, tsl])
                    for h in range(H):
                        if h == 0:
                            src = d_recip[0:1, tsl]
                        else:
                            nc.sync.dma_start(d_row[h][:],
                                              d_recip[h:h + 1, tsl])
                            src = d_row[h][:]
                        nc.gpsimd.partition_broadcast(rbc[:, h, :], src)
                    with tc.tile_pool(name=f"yn{tb}", bufs=4) as yn_pool:
                        for pair in range(H // 2):
                            for j, h in ((0, 2 * pair), (1, 2 * pair + 1)):
                                yn = yn_pool.tile([D, TB], BF16, tag="yn",
                                                  name="yn")
                                nc.vector.tensor_mul(
                                    yn[0:D, :], yu[h][0:D, tsl],
                                    rbc[:, h, :])
                                nc.gpsimd.dma_start(
                                    yT_pair[pair][j * D:(j + 1) * D, tsl],
                                    yn[0:D, :])

                    # ---- phase 3 for this tb's t-chunks ----
                    with (tc.tile_pool(name=f"osb{tb}", bufs=2) as o_pool):
                        for tch in range(tb * (ntc // ntb),
                                         (tb + 1) * (ntc // ntb)):
                            po = psA.tile([P, 2 * TB], F32, tag="s",
                                          name="po")
                            for j in range(H // 2):
                                nc.tensor.matmul(
                                    po[:, 0:C],
                                    yT_pair[j][:, tch * P:(tch + 1) * P],
                                    w_sb["wp"][:, j],
                                    start=(j == 0), stop=(j == H // 2 - 1))
                            o_sb = o_pool.tile([P, C], F32, tag="o_sb")
                            nc.vector.scalar_tensor_tensor(
                                o_sb[:], po[:, 0:C], 0.0, bp_bc[:],
                                op0=ALU.add, op1=ALU.add)
                            nc.sync.dma_start(out[tch * P:(tch + 1) * P, :],
                                              o_sb[:])

        # small rotating SBUF pools for attention tiles
        _am_pool = {}

        def pp_at(tc_, pair, tb, i):
            return _am_pool["at"].tile([P, 2 * TB], BF16, tag="at", name="at")

        def pp_am(tc_, pair, tb, i):
            return _am_pool["am"].tile([P, 2 * TB], BF16, tag="am", name="am")

        with (tc.tile_pool(name="atp", bufs=4) as atp,
              tc.tile_pool(name="amp", bufs=4) as amp):
            _am_pool["at"] = atp
            _am_pool["am"] = amp
            if loop_reps > 1:
                ET = mybir.EngineType
                with tc.For_i(0, loop_reps, 1,
                              hint_engines=(ET.PE, ET.DVE, ET.Activation,
                                            ET.Pool, ET.SP)):
                    body()
            else:
                body()

    nc.compile()
    return nc


def shard_inputs(x, adj_matrix, w_qkv, b_qkv, w_proj, b_proj,
                 t_full=T, t_local=TL):
    """Host-side shard/layout prep. Core c handles (b, th) = divmod(c, 2)."""
    wq = np.ascontiguousarray(w_qkv[:, 0:C]).astype(nbf16)
    wk = np.ascontiguousarray(w_qkv[:, C:2 * C]).astype(nbf16)
    wv = np.ascontiguousarray(w_qkv[:, 2 * C:3 * C]).astype(nbf16)
    wp = np.ascontiguousarray(w_proj).astype(nbf16)
    bq = np.ascontiguousarray(b_qkv[0:C]).astype(np.float32)
    bk = np.ascontiguousarray(b_qkv[C:2 * C]).astype(np.float32)
    bv = np.ascontiguousarray(b_qkv[2 * C:3 * C]).astype(np.float32)
    # y_normalized = y0/d + bv, so out = (y0/d) @ wp + (bp + bv @ wp)
    bp = np.ascontiguousarray(b_proj + bv @ w_proj).astype(np.float32)[None]
    in_maps = []
    n_th = t_full // t_local
    nsc = t_full // P
    for core in range(B * n_th):
        b, th = divmod(core, n_th)
        xTb = np.ascontiguousarray(x[b, :t_full].T).astype(nbf16)
        tsl = slice(th * t_local, (th + 1) * t_local)
        mT = adj_matrix[b, :t_full, :t_full].T[:, tsl]  # [s, t] bool
        mvals = np.where(mT, 1.0, 0.0).astype(np.float32)
        for i in DVE_I:
            rs = slice(i * P, (i + 1) * P)
            mvals[rs] = np.where(mT[rs], SCH_ON, SCH_OFF)
        in_maps.append({
            "xT": xTb,
            "xTq": np.ascontiguousarray(xTb[:, tsl]),
            "maskT": np.ascontiguousarray(mvals).astype(nbf16),
            "wq": wq, "wk": wk, "wv": wv, "wp": wp,
            "bq": bq, "bk": bk, "bp": bp,
        })
    return in_maps


_PROGRAM_CACHE = {}


def _get_program(key=(T, TL, 1)):
    if key not in _PROGRAM_CACHE:
        _PROGRAM_CACHE[key] = build_program(t_full=key[0], t_local=key[1],
                                            loop_reps=key[2])
    return _PROGRAM_CACHE[key]


def kernel(**inputs):
    x = np.asarray(inputs["x"])
    adj = np.asarray(inputs["adj_matrix"])
    nc = _get_program()
    in_maps = shard_inputs(x, adj, np.asarray(inputs["w_qkv"]),
                           np.asarray(inputs["b_qkv"]),
                           np.asarray(inputs["w_proj"]),
                           np.asarray(inputs["b_proj"]))
    res = run_bass_kernel_spmd(nc, in_maps, list(range(NCORES)))
    out = np.empty((B, T, C), dtype=np.float32)
    for core in range(NCORES):
        b, th = divmod(core, 2)
        out[b, th * TL:(th + 1) * TL, :] = res.results[core]["out"]
    return out


# revision 16
# speedup vs baseline: 1.0321x; 1.0321x over previous
"""Sparse (graph-masked) multi-head attention on 8 Trainium2 NeuronCores.

Reference computation (fp32, single device):
    qkv = x @ w_qkv + b_qkv ; split heads (H=8, D=64)
    scores = q k^T / sqrt(D), masked by adj_matrix (True=attend)
    y = softmax(scores) @ v ; out = y @ w_proj + b_proj

Sharding: core = (batch b, query-half th).  Each core owns queries
t in [th*1024, (th+1)*1024) of batch b and produces out[b, that slice, :].
No cross-core communication.

Engine strategy (v2): the kernel is elementwise-bound (exp on ACT,
mask-mul on DVE over H*TL*T = 16.8M elements/core), so:
  * 13/16 key-chunks: ACT exp (with a mean-compensation bias, see below)
    then DVE mask-multiply (bf16 2x mode, broadcast AP).
  * 3/16 key-chunks (DVE_I): one fused DVE scalar_tensor_tensor computes
    round(scores*A + maskS) -> int16, whose bits ARE the bf16 of
    g*exp(scores/sqrt(D)) masked (Schraudolph exponent trick).  maskS
    rows hold 16256 (=127<<7, attend) or 4096 (masked -> 2^-95).
    The common factor g=2^E[log2((1+f)/2^f)] is applied to the ACT
    chunks as exp bias ln(g) so both paths are mean-consistent; g
    cancels row-wise in softmax.
  * softmax denominators: v_aug has a per-head ones-column at local
    column h, so head h's denominator accumulates on PSUM lane h while
    y occupies lanes 8:72.  One [72,TB] copy evacuates y+d together;
    tiny DMAs gather all 8 d-rows into one [8,TL] tile for a single
    batched DVE reciprocal (vs 16 serial [1,512] reciprocals).
  * q/k PSUM evac on ACT (Identity + per-partition bias AP) to offload
    DVE; v evac keeps the fused bias stt on DVE.
  * phase-1 matmuls are interleaved into early attention pairs' PE idle
    slots; PSUM: psA 2x2 banks + psY 2x1 + ps1 2x1 = 8 banks.
"""

import numpy as np
import ml_dtypes

import concourse.bass as bass
import concourse.mybir as mybir
import concourse.tile as tile
from concourse import bacc
from concourse.bass_utils import run_bass_kernel_spmd

BF16 = mybir.dt.bfloat16
F32 = mybir.dt.float32
I16 = mybir.dt.int16
nbf16 = ml_dtypes.bfloat16

B, T, C, H = 4, 2048, 512, 8
D = C // H          # 64
P = 128
NCORES = 8
TL = T // 2         # queries per core
SCALE = 1.0 / float(np.sqrt(D))
LOG2E = float(np.log2(np.e))

# Schraudolph constants (bf16 = top 16 bits of f32; 7 mantissa bits)
SCH_A = SCALE * LOG2E * 128.0          # scores * A + maskS -> i16 bits
SCH_ON = 16256.0                       # 127 << 7 (attend)
SCH_OFF = 4096.0                       # -> 2^-95 ~ 0   (masked)
# mean multiplicative bias of the linear-mantissa approx: g = 2^c
SCH_C = 0.05730496                     # E[log2((1+f)/2^f)]
ACT_BIAS = float(np.log(2.0 ** SCH_C))  # exp(x + ln g) on ACT chunks
DVE_I = (13, 14, 15)                   # s-chunks computed on DVE

AF = mybir.ActivationFunctionType
ALU = mybir.AluOpType

VW = D + 8          # v columns per head: 64 dims + 8 ones-slot lanes
NB = 512            # one PSUM bank of f32
TB = 512            # t-block per attention pass


def build_program(t_full=T, t_local=TL, loop_reps=1, num_devices=NCORES):
    nkc = C // P                # contraction chunks over C
    nsc = t_full // P           # key/s chunks
    ntc = t_local // P          # output t chunks
    ntb = t_local // TB

    nc = bacc.Bacc("TRN2", target_bir_lowering=False, debug=False,
                   num_devices=num_devices)

    xT = nc.dram_tensor("xT", [C, t_full], BF16, kind="ExternalInput").ap()
    xTq = nc.dram_tensor("xTq", [C, t_local], BF16, kind="ExternalInput").ap()
    maskT = nc.dram_tensor("maskT", [t_full, t_local], BF16,
                           kind="ExternalInput").ap()
    wq = nc.dram_tensor("wq", [C, C], BF16, kind="ExternalInput").ap()
    wk = nc.dram_tensor("wk", [C, C], BF16, kind="ExternalInput").ap()
    wv = nc.dram_tensor("wv", [C, C], BF16, kind="ExternalInput").ap()
    wp = nc.dram_tensor("wp", [C, C], BF16, kind="ExternalInput").ap()
    bq = nc.dram_tensor("bq", [C], F32, kind="ExternalInput").ap()
    bk = nc.dram_tensor("bk", [C], F32, kind="ExternalInput").ap()
    bv = nc.dram_tensor("bv", [1, C], F32, kind="ExternalInput").ap()
    bp = nc.dram_tensor("bp", [1, C], F32, kind="ExternalInput").ap()
    out = nc.dram_tensor("out", [t_local, C], F32, kind="ExternalOutput").ap()

    with tile.TileContext(nc) as tc:
        def body():
            with (tc.tile_pool(name="persist", bufs=1) as pp,
                  tc.tile_pool(name="ps1", bufs=2, space="PSUM") as ps1,
                  tc.tile_pool(name="psA", bufs=2, space="PSUM") as psA,
                  tc.tile_pool(name="psY", bufs=1, space="PSUM") as psY):
                # ---- input loads ----
                xT_sb = pp.tile([P, nkc, t_full], BF16, tag="xT")
                nc.sync.dma_start(
                    xT_sb[:], xT.rearrange("(k p) t -> p k t", p=P))
                xTq_sb = pp.tile([P, nkc, t_local], BF16, tag="xTq")
                nc.sync.dma_start(
                    xTq_sb[:], xTq.rearrange("(k p) t -> p k t", p=P))
                mask_sb = pp.tile([P, nsc, t_local], BF16, tag="mask")
                mask_r = maskT.rearrange("(i p) t -> p i t", p=P)
                ngrp = min(4, nsc)
                for g in range(ngrp):
                    gs = nsc // ngrp
                    nc.gpsimd.dma_start(mask_sb[:, g * gs:(g + 1) * gs],
                                        mask_r[:, g * gs:(g + 1) * gs])
                w_sb = {}
                for name, w in (("wq", wq), ("wk", wk), ("wv", wv),
                                ("wp", wp)):
                    w_sb[name] = pp.tile([P, nkc, C], BF16, tag=name,
                                         name=name)
                    nc.sync.dma_start(
                        w_sb[name][:], w.rearrange("(k p) c -> p k c", p=P))
                bq_sb = pp.tile([P, nkc], F32, tag="bq")
                nc.sync.dma_start(bq_sb[:], bq.rearrange("(j p) -> p j", p=P))
                bk_sb = pp.tile([P, nkc], F32, tag="bk")
                nc.sync.dma_start(bk_sb[:], bk.rearrange("(j p) -> p j", p=P))
                bv_row = pp.tile([1, C], F32, tag="bv_row")
                nc.sync.dma_start(bv_row[:], bv[:])
                bp_row = pp.tile([1, C], F32, tag="bp_row")
                nc.sync.dma_start(bp_row[:], bp[:])
                bv_bc = pp.tile([P, C], F32, tag="bv_bc")
                nc.gpsimd.partition_broadcast(bv_bc[:], bv_row[:])
                bp_bc = pp.tile([P, C], F32, tag="bp_bc")
                nc.gpsimd.partition_broadcast(bp_bc[:], bp_row[:])

                # ---- persistent compute tiles ----
                qT_sb = pp.tile([P, nkc, t_local], BF16, tag="qT")
                kT_sb = pp.tile([P, nkc, t_full], BF16, tag="kT")
                v_sb = pp.tile([P, nsc, H * VW], BF16, tag="v")
                yu = [pp.tile([72, t_local], BF16, tag=f"yu{h}",
                              name=f"yu{h}") for h in range(H)]
                yT_pair = [pp.tile([P, t_local], BF16, tag=f"yTp{j}",
                                   name=f"yTp{j}") for j in range(H // 2)]
                d_all = pp.tile([8, t_local], BF16, tag="d_all")
                d_recip = pp.tile([8, t_local], BF16, tag="d_recip")
                d_row = [pp.tile([1, TB], BF16, tag=f"dr{h}",
                                 name=f"dr{h}") for h in range(H)]
                rbc = pp.tile([64, H, TB], BF16, tag="rbc")
                cbias = pp.tile([P, 1], F32, tag="cbias")
                nc.vector.memset(cbias[:], ACT_BIAS)

                # v_aug ones/zero slots: head h occupies v_sb cols
                # [VW*h, VW*(h+1)); local cols 0:64 are the v dims, cols
                # 64:72 d-slots (ones at local col 64+h -> PSUM lane 64+h;
                # engine ops need 32-aligned partition bases, so y keeps
                # lanes 0:64 and d sits above).
                vz = v_sb[:].rearrange("p i (h w) -> p i h w", h=H, w=VW)
                nc.vector.memset(vz[:, :, :, D:VW], 0.0)
                for h in range(H):
                    nc.vector.memset(
                        vz[:, :, h, D + h:D + h + 1], 1.0)

                # ---- phase-1 emit helpers (512-wide PSUM slices) ----
                def nslices(total):
                    return [slice(n, min(n + NB, total))
                            for n in range(0, total, NB)]

                def emit_q(j):
                    for sl in nslices(t_local):
                        pq = ps1.tile([P, NB], F32, tag="p1", name="pq")
                        for k in range(nkc):
                            nc.tensor.matmul(
                                pq[:], w_sb["wq"][:, k, j * P:(j + 1) * P],
                                xTq_sb[:, k, sl],
                                start=(k == 0), stop=(k == nkc - 1))
                        nc.scalar.activation(qT_sb[:, j, sl], pq[:],
                                             AF.Identity,
                                             bias=bq_sb[:, j:j + 1],
                                             scale=1.0)

                def emit_k(j, sls=None):
                    for sl in (sls or nslices(t_full)):
                        pk = ps1.tile([P, NB], F32, tag="p1", name="pk")
                        for k in range(nkc):
                            nc.tensor.matmul(
                                pk[:], w_sb["wk"][:, k, j * P:(j + 1) * P],
                                xT_sb[:, k, sl],
                                start=(k == 0), stop=(k == nkc - 1))
                        nc.scalar.activation(kT_sb[:, j, sl], pk[:],
                                             AF.Identity,
                                             bias=bk_sb[:, j:j + 1],
                                             scale=1.0)

                def emit_v(i):
                    pv = ps1.tile([P, C], F32, tag="p1", name="pv")
                    for k in range(nkc):
                        nc.tensor.matmul(
                            pv[:], xT_sb[:, k, i * P:(i + 1) * P],
                            w_sb["wv"][:, k], start=(k == 0),
                            stop=(k == nkc - 1))
                    v_dst = v_sb[:, i].rearrange(
                        "p (h w) -> p h w", w=VW)[:, :, 0:D]
                    nc.vector.scalar_tensor_tensor(
                        v_dst, pv[:].rearrange("p (h d) -> p h d", d=D),
                        0.0, bv_bc[:].rearrange("p (h d) -> p h d", d=D),
                        op0=ALU.add, op1=ALU.add)

                # ---- lead-in: what attention pair 0 needs first ----
                emit_q(0)
                emit_k(0)
                for i in range(6):
                    emit_v(i)

                # extras: phase-1 work interleaved into EARLIER pairs'
                # attention loops (pair p's q/k must be done before pair
                # p's first scores matmul, so emit them during pair p-1).
                extras = {
                    (0, 0): ([lambda i=i: emit_v(i) for i in range(6, nsc)]
                             + [lambda: emit_q(1)]
                             + [lambda sl=sl: emit_k(1, [sl])
                                for sl in nslices(t_full)]),
                    (0, 1): ([lambda: emit_q(2)]
                             + [lambda sl=sl: emit_k(2, [sl])
                                for sl in nslices(t_full)]),
                    (0, 2): ([lambda: emit_q(3)]
                             + [lambda sl=sl: emit_k(3, [sl])
                                for sl in nslices(t_full)]),
                }

                # ---- phase 2: attention (tb outer, pair inner) ----
                for tb in range(ntb):
                    tsl = slice(tb * TB, (tb + 1) * TB)
                    for pair in range(H // 2):
                        h0, h1 = 2 * pair, 2 * pair + 1
                        todo = list(extras.get((tb, pair), []))
                        py0 = psY.tile([72, TB], F32, tag="py0", name="py0")
                        py1 = psY.tile([72, TB], F32, tag="py1", name="py1")
                        ps_next = {}

                        def emit_scores(i):
                            ps = psA.tile([P, 2 * TB], F32, tag="s",
                                          name="ps")
                            ps_next[i] = ps
                            nc.tensor.matmul(
                                ps[:, 0:TB],
                                kT_sb[0:D, pair, i * P:(i + 1) * P],
                                qT_sb[0:D, pair, tsl],
                                start=True, stop=True, tile_position=(0, 0))
                            nc.tensor.matmul(
                                ps[:, TB:2 * TB],
                                kT_sb[D:P, pair, i * P:(i + 1) * P],
                                qT_sb[D:P, pair, tsl],
                                start=True, stop=True, tile_position=(D, 0))

                        emit_scores(0)
                        for i in range(nsc):
                            if i + 1 < nsc:
                                emit_scores(i + 1)
                            # one interleaved phase-1 item per chunk
                            if todo:
                                todo.pop(0)()
                            ps = ps_next.pop(i)
                            if i in DVE_I:
                                maskS_bc = mask_sb[:, i, tsl].rearrange(
                                    "p (o n) -> p o n",
                                    o=1).broadcast_to([P, 2, TB])
                                am = pp_am(tc, pair, tb, i)
                                nc.vector.scalar_tensor_tensor(
                                    am[:].bitcast(I16).rearrange(
                                        "p (g n) -> p g n", g=2),
                                    ps[:].rearrange("p (g n) -> p g n", g=2),
                                    SCH_A, maskS_bc,
                                    op0=ALU.mult, op1=ALU.add)
                            else:
                                at = pp_at(tc, pair, tb, i)
                                nc.scalar.activation(at[:], ps[:], AF.Exp,
                                                     scale=SCALE,
                                                     bias=cbias[:, 0:1])
                                mask_bc = mask_sb[:, i, tsl].rearrange(
                                    "p (o n) -> p o n",
                                    o=1).broadcast_to([P, 2, TB])
                                am = pp_am(tc, pair, tb, i)
                                nc.vector.tensor_mul(
                                    am[:].rearrange("p (g n) -> p g n", g=2),
                                    at[:].rearrange("p (g n) -> p g n", g=2),
                                    mask_bc)
                            nc.tensor.matmul(
                                py0[:], v_sb[:, i, h0 * VW:(h0 + 1) * VW],
                                am[:, 0:TB], start=(i == 0),
                                stop=(i == nsc - 1))
                            nc.tensor.matmul(
                                py1[:], v_sb[:, i, h1 * VW:(h1 + 1) * VW],
                                am[:, TB:2 * TB], start=(i == 0),
                                stop=(i == nsc - 1))
                        while todo:
                            todo.pop(0)()
                        # evacuate y (lanes 8:72) + d (lane h) together
                        nc.vector.tensor_copy(yu[h0][:, tsl], py0[:])
                        nc.vector.tensor_copy(yu[h1][:, tsl], py1[:])

                    # ---- per-tb: batched reciprocal + normalize ----
                    for h in range(H):
                        nc.gpsimd.dma_start(d_all[h:h + 1, tsl],
                                            yu[h][D + h:D + h + 1, tsl])
                    with nc.allow_low_precision(
                            reason="softmax denom ~1e3, bf16 ok"):
                        nc.vector.reciprocal(d_recip[:, tsl],
                                             d_all[:, tsl])
                    for h in range(H):
                        if h == 0:
                            src = d_recip[0:1, tsl]
                        else:
                            nc.sync.dma_start(d_row[h][:],
                                              d_recip[h:h + 1, tsl])
                            src = d_row[h][:]
                        nc.gpsimd.partition_broadcast(rbc[:, h, :], src)
                    with tc.tile_pool(name=f"yn{tb}", bufs=4) as yn_pool:
                        for pair in range(H // 2):
                            for j, h in ((0, 2 * pair), (1, 2 * pair + 1)):
                                yn = yn_pool.tile([D, TB], BF16, tag="yn",
                                                  name="yn")
                                nc.vector.tensor_mul(
                                    yn[0:D, :], yu[h][0:D, tsl],
                                    rbc[:, h, :])
                                nc.gpsimd.dma_start(
                                    yT_pair[pair][j * D:(j + 1) * D, tsl],
                                    yn[0:D, :])

                    # ---- phase 3 for this tb's t-chunks ----
                    with (tc.tile_pool(name=f"osb{tb}", bufs=2) as o_pool):
                        for tch in range(tb * (ntc // ntb),
                                         (tb + 1) * (ntc // ntb)):
                            po = ps1.tile([P, C], F32, tag="p1", name="po")
                            for j in range(H // 2):
                                nc.tensor.matmul(
                                    po[:],
                                    yT_pair[j][:, tch * P:(tch + 1) * P],
                                    w_sb["wp"][:, j],
                                    start=(j == 0), stop=(j == H // 2 - 1))
                            o_sb = o_pool.tile([P, C], F32, tag="o_sb")
                            nc.vector.scalar_tensor_tensor(
                                o_sb[:], po[:], 0.0, bp_bc[:],
                                op0=ALU.add, op1=ALU.add)
                            nc.sync.dma_start(out[tch * P:(tch + 1) * P, :],
                                              o_sb[:])

        # small rotating SBUF pools for attention tiles
        _am_pool = {}

        def pp_at(tc_, pair, tb, i):
            return _am_pool["at"].tile([P, 2 * TB], BF16, tag="at", name="at")

        def pp_am(tc_, pair, tb, i):
            return _am_pool["am"].tile([P, 2 * TB], BF16, tag="am", name="am")

        with (tc.tile_pool(name="atp", bufs=4) as atp,
              tc.tile_pool(name="amp", bufs=4) as amp):
            _am_pool["at"] = atp
            _am_pool["am"] = amp
            if loop_reps > 1:
                ET = mybir.EngineType
                with tc.For_i(0, loop_reps, 1,
                              hint_engines=(ET.PE, ET.DVE, ET.Activation,
                                            ET.Pool, ET.SP)):
                    body()
            else:
                body()

    nc.compile()
    return nc


def shard_inputs(x, adj_matrix, w_qkv, b_qkv, w_proj, b_proj,
                 t_full=T, t_local=TL):
    """Host-side shard/layout prep. Core c handles (b, th) = divmod(c, 2)."""
    wq = np.ascontiguousarray(w_qkv[:, 0:C]).astype(nbf16)
    wk = np.ascontiguousarray(w_qkv[:, C:2 * C]).astype(nbf16)
    wv = np.ascontiguousarray(w_qkv[:, 2 * C:3 * C]).astype(nbf16)
    wp = np.ascontiguousarray(w_proj).astype(nbf16)
    bq = np.ascontiguousarray(b_qkv[0:C]).astype(np.float32)
    bk = np.ascontiguousarray(b_qkv[C:2 * C]).astype(np.float32)
    bv = np.ascontiguousarray(b_qkv[2 * C:3 * C]).astype(np.float32)[None]
    bp = np.ascontiguousarray(b_proj).astype(np.float32)[None]
    in_maps = []
    n_th = t_full // t_local
    nsc = t_full // P
    for core in range(B * n_th):
        b, th = divmod(core, n_th)
        xTb = np.ascontiguousarray(x[b, :t_full].T).astype(nbf16)
        tsl = slice(th * t_local, (th + 1) * t_local)
        mT = adj_matrix[b, :t_full, :t_full].T[:, tsl]  # [s, t] bool
        mvals = np.where(mT, 1.0, 0.0).astype(np.float32)
        for i in DVE_I:
            rs = slice(i * P, (i + 1) * P)
            mvals[rs] = np.where(mT[rs], SCH_ON, SCH_OFF)
        in_maps.append({
            "xT": xTb,
            "xTq": np.ascontiguousarray(xTb[:, tsl]),
            "maskT": np.ascontiguousarray(mvals).astype(nbf16),
            "wq": wq, "wk": wk, "wv": wv, "wp": wp,
            "bq": bq, "bk": bk, "bv": bv, "bp": bp,
        })
    return in_maps


_PROGRAM_CACHE = {}


def _get_program(key=(T, TL, 1)):
    if key not in _PROGRAM_CACHE:
        _PROGRAM_CACHE[key] = build_program(t_full=key[0], t_local=key[1],
                                            loop_reps=key[2])
    return _PROGRAM_CACHE[key]


def kernel(**inputs):
    x = np.asarray(inputs["x"])
    adj = np.asarray(inputs["adj_matrix"])
    nc = _get_program()
    in_maps = shard_inputs(x, adj, np.asarray(inputs["w_qkv"]),
                           np.asarray(inputs["b_qkv"]),
                           np.asarray(inputs["w_proj"]),
                           np.asarray(inputs["b_proj"]))
    res = run_bass_kernel_spmd(nc, in_maps, list(range(NCORES)))
    out = np.empty((B, T, C), dtype=np.float32)
    for core in range(NCORES):
        b, th = divmod(core, 2)
        out[b, th * TL:(th + 1) * TL, :] = res.results[core]["out"]
    return out


# revision 18
# speedup vs baseline: 1.0372x; 1.0050x over previous
"""Sparse (graph-masked) multi-head attention on 8 Trainium2 NeuronCores.

Reference computation (fp32, single device):
    qkv = x @ w_qkv + b_qkv ; split heads (H=8, D=64)
    scores = q k^T / sqrt(D), masked by adj_matrix (True=attend)
    y = softmax(scores) @ v ; out = y @ w_proj + b_proj

Sharding: core = (batch b, query-half th).  Each core owns queries
t in [th*1024, (th+1)*1024) of batch b and produces out[b, that slice, :].
No cross-core communication.

Engine strategy (v2): the kernel is elementwise-bound (exp on ACT,
mask-mul on DVE over H*TL*T = 16.8M elements/core), so:
  * 13/16 key-chunks: ACT exp (with a mean-compensation bias, see below)
    then DVE mask-multiply (bf16 2x mode, broadcast AP).
  * 3/16 key-chunks (DVE_I): one fused DVE scalar_tensor_tensor computes
    round(scores*A + maskS) -> int16, whose bits ARE the bf16 of
    g*exp(scores/sqrt(D)) masked (Schraudolph exponent trick).  maskS
    rows hold 16256 (=127<<7, attend) or 4096 (masked -> 2^-95).
    The common factor g=2^E[log2((1+f)/2^f)] is applied to the ACT
    chunks as exp bias ln(g) so both paths are mean-consistent; g
    cancels row-wise in softmax.
  * softmax denominators: v_aug has a per-head ones-column at local
    column 64+h, so head h's denominator accumulates on PSUM lane 64+h
    while y occupies lanes 0:64.  One [72,TB] copy evacuates y+d at once;
    tiny DMAs gather all 8 d-rows into one [8,TL] tile for a single
    batched DVE reciprocal (vs 16 serial [1,512] reciprocals).
  * q/k PSUM evac on ACT (Identity + per-partition bias AP) to offload
    DVE; v evac is a pure ACT copy (b_qkv's v-bias is folded into bp on
    the host: y_norm = y0/d + bv, so out gains the constant bv @ w_proj).
  * attn@v matmuls are emitted one chunk DEFERRED so the in-order PE
    queue never blocks scores(i+1) behind an attn@v waiting on the
    exp->mask chain (stream: ..., s(i+1), a(i-1), s(i+2), a(i), ...).
  * phase-1 matmuls are interleaved into early attention pairs' PE idle
    slots; PSUM: psA 2x2 banks + psY 2x1 + ps1 2x1 = 8 banks.
"""

import numpy as np
import ml_dtypes

import concourse.bass as bass
import concourse.mybir as mybir
import concourse.tile as tile
from concourse import bacc
from concourse.bass_utils import run_bass_kernel_spmd

BF16 = mybir.dt.bfloat16
F32 = mybir.dt.float32
I16 = mybir.dt.int16
nbf16 = ml_dtypes.bfloat16

B, T, C, H = 4, 2048, 512, 8
D = C // H          # 64
P = 128
NCORES = 8
TL = T // 2         # queries per core
SCALE = 1.0 / float(np.sqrt(D))
LOG2E = float(np.log2(np.e))

# Schraudolph constants (bf16 = top 16 bits of f32; 7 mantissa bits)
SCH_A = SCALE * LOG2E * 128.0          # scores * A + maskS -> i16 bits
SCH_ON = 16256.0                       # 127 << 7 (attend)
SCH_OFF = 4096.0                       # -> 2^-95 ~ 0   (masked)
# mean multiplicative bias of the linear-mantissa approx: g = 2^c
SCH_C = 0.05730496                     # E[log2((1+f)/2^f)]
ACT_BIAS = float(np.log(2.0 ** SCH_C))  # exp(x + ln g) on ACT chunks
DVE_I = (13, 14, 15)                   # s-chunks computed on DVE

AF = mybir.ActivationFunctionType
ALU = mybir.AluOpType

VW = D + 8          # v columns per head: 64 dims + 8 ones-slot lanes
NB = 512            # one PSUM bank of f32
TB = 512            # t-block per attention pass


def build_program(t_full=T, t_local=TL, loop_reps=1, num_devices=NCORES):
    nkc = C // P                # contraction chunks over C
    nsc = t_full // P           # key/s chunks
    ntc = t_local // P          # output t chunks
    ntb = t_local // TB

    nc = bacc.Bacc("TRN2", target_bir_lowering=False, debug=False,
                   num_devices=num_devices)

    xT = nc.dram_tensor("xT", [C, t_full], BF16, kind="ExternalInput").ap()
    xTq = nc.dram_tensor("xTq", [C, t_local], BF16, kind="ExternalInput").ap()
    maskT = nc.dram_tensor("maskT", [t_full, t_local], BF16,
                           kind="ExternalInput").ap()
    wq = nc.dram_tensor("wq", [C, C], BF16, kind="ExternalInput").ap()
    wk = nc.dram_tensor("wk", [C, C], BF16, kind="ExternalInput").ap()
    wv = nc.dram_tensor("wv", [C, C], BF16, kind="ExternalInput").ap()
    wp = nc.dram_tensor("wp", [C, C], BF16, kind="ExternalInput").ap()
    bq = nc.dram_tensor("bq", [C], F32, kind="ExternalInput").ap()
    bk = nc.dram_tensor("bk", [C], F32, kind="ExternalInput").ap()
    bp = nc.dram_tensor("bp", [1, C], F32, kind="ExternalInput").ap()
    out = nc.dram_tensor("out", [t_local, C], F32, kind="ExternalOutput").ap()

    with tile.TileContext(nc) as tc:
        def body():
            with (tc.tile_pool(name="persist", bufs=1) as pp,
                  tc.tile_pool(name="ps1", bufs=2, space="PSUM") as ps1,
                  tc.tile_pool(name="psA", bufs=2, space="PSUM") as psA,
                  tc.tile_pool(name="psY", bufs=1, space="PSUM") as psY):
                # ---- input loads ----
                xT_sb = pp.tile([P, nkc, t_full], BF16, tag="xT")
                nc.sync.dma_start(
                    xT_sb[:], xT.rearrange("(k p) t -> p k t", p=P))
                xTq_sb = pp.tile([P, nkc, t_local], BF16, tag="xTq")
                nc.sync.dma_start(
                    xTq_sb[:], xTq.rearrange("(k p) t -> p k t", p=P))
                mask_sb = pp.tile([P, nsc, t_local], BF16, tag="mask")
                mask_r = maskT.rearrange("(i p) t -> p i t", p=P)
                ngrp = min(4, nsc)
                for g in range(ngrp):
                    gs = nsc // ngrp
                    nc.gpsimd.dma_start(mask_sb[:, g * gs:(g + 1) * gs],
                                        mask_r[:, g * gs:(g + 1) * gs])
                w_sb = {}
                for name, w in (("wq", wq), ("wk", wk), ("wv", wv),
                                ("wp", wp)):
                    w_sb[name] = pp.tile([P, nkc, C], BF16, tag=name,
                                         name=name)
                    nc.sync.dma_start(
                        w_sb[name][:], w.rearrange("(k p) c -> p k c", p=P))
                bq_sb = pp.tile([P, nkc], F32, tag="bq")
                nc.sync.dma_start(bq_sb[:], bq.rearrange("(j p) -> p j", p=P))
                bk_sb = pp.tile([P, nkc], F32, tag="bk")
                nc.sync.dma_start(bk_sb[:], bk.rearrange("(j p) -> p j", p=P))
                bp_row = pp.tile([1, C], F32, tag="bp_row")
                nc.sync.dma_start(bp_row[:], bp[:])
                bp_bc = pp.tile([P, C], F32, tag="bp_bc")
                nc.gpsimd.partition_broadcast(bp_bc[:], bp_row[:])

                # ---- persistent compute tiles ----
                qT_sb = pp.tile([P, nkc, t_local], BF16, tag="qT")
                kT_sb = pp.tile([P, nkc, t_full], BF16, tag="kT")
                v_sb = pp.tile([P, nsc, H * VW], BF16, tag="v")
                yu = [pp.tile([72, t_local], BF16, tag=f"yu{h}",
                              name=f"yu{h}") for h in range(H)]
                yT_pair = [pp.tile([P, t_local], BF16, tag=f"yTp{j}",
                                   name=f"yTp{j}") for j in range(H // 2)]
                d_all = pp.tile([8, t_local], BF16, tag="d_all")
                d_recip = pp.tile([8, t_local], BF16, tag="d_recip")
                d_row = [pp.tile([1, TB], BF16, tag=f"dr{h}",
                                 name=f"dr{h}") for h in range(H)]
                rbc = pp.tile([64, H, TB], BF16, tag="rbc")
                cbias = pp.tile([P, 1], F32, tag="cbias")
                nc.vector.memset(cbias[:], ACT_BIAS)

                # v_aug ones/zero slots: head h occupies v_sb cols
                # [VW*h, VW*(h+1)); local cols 0:64 are the v dims, cols
                # 64:72 d-slots (ones at local col 64+h -> PSUM lane 64+h;
                # engine ops need 32-aligned partition bases, so y keeps
                # lanes 0:64 and d sits above).
                vz = v_sb[:].rearrange("p i (h w) -> p i h w", h=H, w=VW)
                nc.vector.memset(vz[:, :, :, D:VW], 0.0)
                for h in range(H):
                    nc.vector.memset(
                        vz[:, :, h, D + h:D + h + 1], 1.0)

                # ---- phase-1 emit helpers (512-wide PSUM slices) ----
                def nslices(total):
                    return [slice(n, min(n + NB, total))
                            for n in range(0, total, NB)]

                def emit_q(j):
                    for sl in nslices(t_local):
                        pq = ps1.tile([P, NB], F32, tag="p1", name="pq")
                        for k in range(nkc):
                            nc.tensor.matmul(
                                pq[:], w_sb["wq"][:, k, j * P:(j + 1) * P],
                                xTq_sb[:, k, sl],
                                start=(k == 0), stop=(k == nkc - 1))
                        nc.scalar.activation(qT_sb[:, j, sl], pq[:],
                                             AF.Identity,
                                             bias=bq_sb[:, j:j + 1],
                                             scale=1.0)

                def emit_k(j, sls=None):
                    for sl in (sls or nslices(t_full)):
                        pk = ps1.tile([P, NB], F32, tag="p1", name="pk")
                        for k in range(nkc):
                            nc.tensor.matmul(
                                pk[:], w_sb["wk"][:, k, j * P:(j + 1) * P],
                                xT_sb[:, k, sl],
                                start=(k == 0), stop=(k == nkc - 1))
                        nc.scalar.activation(kT_sb[:, j, sl], pk[:],
                                             AF.Identity,
                                             bias=bk_sb[:, j:j + 1],
                                             scale=1.0)

                def emit_v(i):
                    pv = ps1.tile([P, C], F32, tag="p1", name="pv")
                    for k in range(nkc):
                        nc.tensor.matmul(
                            pv[:], xT_sb[:, k, i * P:(i + 1) * P],
                            w_sb["wv"][:, k], start=(k == 0),
                            stop=(k == nkc - 1))
                    v_dst = v_sb[:, i].rearrange(
                        "p (h w) -> p h w", w=VW)[:, :, 0:D]
                    nc.scalar.activation(
                        v_dst, pv[:].rearrange("p (h d) -> p h d", d=D),
                        AF.Identity, bias=0.0, scale=1.0)

                # ---- lead-in: what attention pair 0 needs first ----
                emit_q(0)
                emit_k(0)
                for i in range(6):
                    emit_v(i)

                # extras: phase-1 work interleaved into EARLIER pairs'
                # attention loops (pair p's q/k must be done before pair
                # p's first scores matmul, so emit them during pair p-1).
                extras = {
                    (0, 0): ([lambda i=i: emit_v(i) for i in range(6, nsc)]
                             + [lambda: emit_q(1)]
                             + [lambda sl=sl: emit_k(1, [sl])
                                for sl in nslices(t_full)]),
                    (0, 1): ([lambda: emit_q(2)]
                             + [lambda sl=sl: emit_k(2, [sl])
                                for sl in nslices(t_full)]),
                    (0, 2): ([lambda: emit_q(3)]
                             + [lambda sl=sl: emit_k(3, [sl])
                                for sl in nslices(t_full)]),
                }

                # ---- phase 2: attention (tb outer, pair inner) ----
                for tb in range(ntb):
                    tsl = slice(tb * TB, (tb + 1) * TB)
                    for pair in range(H // 2):
                        h0, h1 = 2 * pair, 2 * pair + 1
                        todo = list(extras.get((tb, pair), []))
                        py0 = psY.tile([72, TB], F32, tag="py0", name="py0")
                        py1 = psY.tile([72, TB], F32, tag="py1", name="py1")
                        ps_next = {}

                        def emit_scores(i):
                            ps = psA.tile([P, 2 * TB], F32, tag="s",
                                          name="ps")
                            ps_next[i] = ps
                            nc.tensor.matmul(
                                ps[:, 0:TB],
                                kT_sb[0:D, pair, i * P:(i + 1) * P],
                                qT_sb[0:D, pair, tsl],
                                start=True, stop=True, tile_position=(0, 0))
                            nc.tensor.matmul(
                                ps[:, TB:2 * TB],
                                kT_sb[D:P, pair, i * P:(i + 1) * P],
                                qT_sb[D:P, pair, tsl],
                                start=True, stop=True, tile_position=(D, 0))

                        emit_scores(0)
                        am_pend = []

                        def emit_attnv(j, amj):
                            nc.tensor.matmul(
                                py0[:], v_sb[:, j, h0 * VW:(h0 + 1) * VW],
                                amj[:, 0:TB], start=(j == 0),
                                stop=(j == nsc - 1))
                            nc.tensor.matmul(
                                py1[:], v_sb[:, j, h1 * VW:(h1 + 1) * VW],
                                amj[:, TB:2 * TB], start=(j == 0),
                                stop=(j == nsc - 1))

                        for i in range(nsc):
                            if i + 1 < nsc:
                                emit_scores(i + 1)
                            # one interleaved phase-1 item per chunk
                            if todo:
                                todo.pop(0)()
                            ps = ps_next.pop(i)
                            if i in DVE_I:
                                maskS_bc = mask_sb[:, i, tsl].rearrange(
                                    "p (o n) -> p o n",
                                    o=1).broadcast_to([P, 2, TB])
                                am = pp_am(tc, pair, tb, i)
                                nc.vector.scalar_tensor_tensor(
                                    am[:].bitcast(I16).rearrange(
                                        "p (g n) -> p g n", g=2),
                                    ps[:].rearrange("p (g n) -> p g n", g=2),
                                    SCH_A, maskS_bc,
                                    op0=ALU.mult, op1=ALU.add)
                            else:
                                at = pp_at(tc, pair, tb, i)
                                nc.scalar.activation(at[:], ps[:], AF.Exp,
                                                     scale=SCALE,
                                                     bias=cbias[:, 0:1])
                                mask_bc = mask_sb[:, i, tsl].rearrange(
                                    "p (o n) -> p o n",
                                    o=1).broadcast_to([P, 2, TB])
                                am = pp_am(tc, pair, tb, i)
                                nc.vector.tensor_mul(
                                    am[:].rearrange("p (g n) -> p g n", g=2),
                                    at[:].rearrange("p (g n) -> p g n", g=2),
                                    mask_bc)
                            am_pend.append((i, am))
                            if i >= 1:
                                emit_attnv(*am_pend.pop(0))
                        while am_pend:
                            emit_attnv(*am_pend.pop(0))
                        while todo:
                            todo.pop(0)()
                        # evacuate y (lanes 8:72) + d (lane h) together
                        nc.vector.tensor_copy(yu[h0][:, tsl], py0[:])
                        nc.vector.tensor_copy(yu[h1][:, tsl], py1[:])

                    # ---- per-tb: batched reciprocal + normalize ----
                    for h in range(H):
                        nc.gpsimd.dma_start(d_all[h:h + 1, tsl],
                                            yu[h][D + h:D + h + 1, tsl])
                    with nc.allow_low_precision(
                            reason="softmax denom ~1e3, bf16 ok"):
                        nc.vector.reciprocal(d_recip[:, tsl],
                                             d_all[:, tsl])
                    for h in range(H):
                        if h == 0:
                            src = d_recip[0:1, tsl]
                        else:
                            nc.sync.dma_start(d_row[h][:],
                                              d_recip[h:h + 1, tsl])
                            src = d_row[h][:]
                        nc.gpsimd.partition_broadcast(rbc[:, h, :], src)
                    with tc.tile_pool(name=f"yn{tb}", bufs=4) as yn_pool:
                        for pair in range(H // 2):
                            for j, h in ((0, 2 * pair), (1, 2 * pair + 1)):
                                yn = yn_pool.tile([D, TB], BF16, tag="yn",
                                                  name="yn")
                                nc.vector.tensor_mul(
                                    yn[0:D, :], yu[h][0:D, tsl],
                                    rbc[:, h, :])
                                nc.gpsimd.dma_start(
                                    yT_pair[pair][j * D:(j + 1) * D, tsl],
                                    yn[0:D, :])

                    # ---- phase 3 for this tb's t-chunks ----
                    with (tc.tile_pool(name=f"osb{tb}", bufs=2) as o_pool):
                        for tch in range(tb * (ntc // ntb),
                                         (tb + 1) * (ntc // ntb)):
                            po = ps1.tile([P, C], F32, tag="p1", name="po")
                            for j in range(H // 2):
                                nc.tensor.matmul(
                                    po[:],
                                    yT_pair[j][:, tch * P:(tch + 1) * P],
                                    w_sb["wp"][:, j],
                                    start=(j == 0), stop=(j == H // 2 - 1))
                            o_sb = o_pool.tile([P, C], F32, tag="o_sb")
                            nc.vector.scalar_tensor_tensor(
                                o_sb[:], po[:], 0.0, bp_bc[:],
                                op0=ALU.add, op1=ALU.add)
                            nc.sync.dma_start(out[tch * P:(tch + 1) * P, :],
                                              o_sb[:])

        # small rotating SBUF pools for attention tiles
        _am_pool = {}

        def pp_at(tc_, pair, tb, i):
            return _am_pool["at"].tile([P, 2 * TB], BF16, tag="at", name="at")

        def pp_am(tc_, pair, tb, i):
            return _am_pool["am"].tile([P, 2 * TB], BF16, tag="am", name="am")

        with (tc.tile_pool(name="atp", bufs=4) as atp,
              tc.tile_pool(name="amp", bufs=4) as amp):
            _am_pool["at"] = atp
            _am_pool["am"] = amp
            if loop_reps > 1:
                ET = mybir.EngineType
                with tc.For_i(0, loop_reps, 1,
                              hint_engines=(ET.PE, ET.DVE, ET.Activation,
                                            ET.Pool, ET.SP)):
                    body()
            else:
                body()

    nc.compile()
    return nc


def shard_inputs(x, adj_matrix, w_qkv, b_qkv, w_proj, b_proj,
                 t_full=T, t_local=TL):
    """Host-side shard/layout prep. Core c handles (b, th) = divmod(c, 2)."""
    wq = np.ascontiguousarray(w_qkv[:, 0:C]).astype(nbf16)
    wk = np.ascontiguousarray(w_qkv[:, C:2 * C]).astype(nbf16)
    wv = np.ascontiguousarray(w_qkv[:, 2 * C:3 * C]).astype(nbf16)
    wp = np.ascontiguousarray(w_proj).astype(nbf16)
    bq = np.ascontiguousarray(b_qkv[0:C]).astype(np.float32)
    bk = np.ascontiguousarray(b_qkv[C:2 * C]).astype(np.float32)
    bv = np.ascontiguousarray(b_qkv[2 * C:3 * C]).astype(np.float32)
    # y_normalized = y0/d + bv, so out = (y0/d) @ wp + (bp + bv @ wp)
    bp = np.ascontiguousarray(b_proj + bv @ w_proj).astype(np.float32)[None]
    in_maps = []
    n_th = t_full // t_local
    nsc = t_full // P
    for core in range(B * n_th):
        b, th = divmod(core, n_th)
        xTb = np.ascontiguousarray(x[b, :t_full].T).astype(nbf16)
        tsl = slice(th * t_local, (th + 1) * t_local)
        mT = adj_matrix[b, :t_full, :t_full].T[:, tsl]  # [s, t] bool
        mvals = np.where(mT, 1.0, 0.0).astype(np.float32)
        for i in DVE_I:
            rs = slice(i * P, (i + 1) * P)
            mvals[rs] = np.where(mT[rs], SCH_ON, SCH_OFF)
        in_maps.append({
            "xT": xTb,
            "xTq": np.ascontiguousarray(xTb[:, tsl]),
            "maskT": np.ascontiguousarray(mvals).astype(nbf16),
            "wq": wq, "wk": wk, "wv": wv, "wp": wp,
            "bq": bq, "bk": bk, "bp": bp,
        })
    return in_maps


_PROGRAM_CACHE = {}


def _get_program(key=(T, TL, 1)):
    if key not in _PROGRAM_CACHE:
        _PROGRAM_CACHE[key] = build_program(t_full=key[0], t_local=key[1],
                                            loop_reps=key[2])
    return _PROGRAM_CACHE[key]


def kernel(**inputs):
    x = np.asarray(inputs["x"])
    adj = np.asarray(inputs["adj_matrix"])
    nc = _get_program()
    in_maps = shard_inputs(x, adj, np.asarray(inputs["w_qkv"]),
                           np.asarray(inputs["b_qkv"]),
                           np.asarray(inputs["w_proj"]),
                           np.asarray(inputs["b_proj"]))
    res = run_bass_kernel_spmd(nc, in_maps, list(range(NCORES)))
    out = np.empty((B, T, C), dtype=np.float32)
    for core in range(NCORES):
        b, th = divmod(core, 2)
        out[b, th * TL:(th + 1) * TL, :] = res.results[core]["out"]
    return out
